# revision 27
# baseline (speedup 1.0000x reference)
"""Trainium2 Bass kernel for MinibatchDiscrimination.

Reference computation (B=256, IN=1024, O=64, K=50):
    M = (x @ T).reshape(B, O, K)
    l1[i,j,o] = sum_k |M[i,o,k] - M[j,o,k]|
    out = concat([x, sum_j exp(-l1) - 1], axis=1)          # [B, IN + O]

Sharding: the O (out_features) dimension is split across the 8 NeuronCores
(8 features per core); x is replicated. Each core computes its [256, 8]
feature block; the host gathers the blocks and concatenates with x.

Per-core pipeline:
  1. PE matmul: M[256, 512] = xT.T @ T_local (bf16 in, f32 PSUM; K padded to
     64 only for this GEMM), cast to bf16 — the canonical value used on BOTH
     sides of the pairwise subtraction, so the diagonal distance is exactly
     zero. -M is staged to DRAM.
  2. All-pairs signed differences are generated by the PE with an affine
     matmul: diff[i, (j,k)] = sum_p lhsT[p,i] * rhs[p,(j,k)] with
     lhsT = [M_o^T (50 k-rows); ones] and rhs = [I50 tiled over j; -M_o row].
     Chunks of 32 j land in PSUM as [128, 4x512] f32 (400 of each 512-col
     bank used: 8 j x 50 k).
  3. Exploiting D[i,j] = D[j,i], the itile-1 blocks only compute j in
     [128,256); the mirrored contribution comes from PE column-sums of the
     itile-0 exp tiles at the end.
  4. Each PSUM chunk takes one of two abs+k-reduce paths (balancing DVE and
     ScalarE): (a) DVE tensor_reduce(add, apply_absolute_value) straight
     from PSUM, or (b) ScalarE Abs-cast to bf16 SBUF (written k-major) +
     dense DVE binary-tree tensor_tensor adds at 2x.
  5. ScalarE exp(-l1) (scale=-1), DVE reduce over j, -1.0, DMA out.
"""

import numpy as np
import ml_dtypes

B = 256
IN_FEATURES = 1024
O_TOTAL = 64
K = 50
K64 = 64
N_CORES = 8
O_LOC = O_TOTAL // N_CORES          # 8 features per core
N_LOC = O_LOC * K64                 # 512 padded M columns per core
P = 128                             # partitions
ITILES = B // P                     # 2 row tiles
CC = IN_FEATURES // P               # 8 contraction chunks
JCHUNK = 32                         # j's per PSUM chunk
JBANK = 8                           # j's per PSUM bank (8*50 = 400 of 512)
QB = JCHUNK // JBANK                # banks per chunk = 4
NCHUNK = B // JCHUNK                # 8 chunks per full block
CPG = 4                             # chunks per tree group
NGROUP = NCHUNK // CPG              # 2 groups per full block
GJ = CPG * JCHUNK                   # 128 j's (= (c,q,j) groups) per tree
DIRECT_EVERY = 5                    # every Nth GROUP takes the DVE-direct path

_cache = {}


def _build_program():
    import concourse.mybir as mybir
    from concourse import bacc, tile
    from concourse.masks import make_identity

    f32 = mybir.dt.float32
    bf16 = mybir.dt.bfloat16
    Alu = mybir.AluOpType
    Act = mybir.ActivationFunctionType

    nc = bacc.Bacc("TRN2", target_bir_lowering=False, debug=False,
                   enable_asserts=False)

    xT_d = nc.dram_tensor("xT", [IN_FEATURES, B], bf16, kind="ExternalInput").ap()
    T_d = nc.dram_tensor("Tl", [IN_FEATURES, N_LOC], bf16, kind="ExternalInput").ap()
    rp_d = nc.dram_tensor("rp", [K + 2, K * B], bf16,
                          kind="ExternalInput").ap()
    feat_d = nc.dram_tensor("feat", [B, O_LOC], f32, kind="ExternalOutput").ap()

    JK = K * B                      # 12800 diff columns per full block
    CH = QB * 512                   # 2048 PSUM elements per chunk (1600 used)
    # ba scratch: 4-chunk level-0 (128 groups x 50) + tree level regions
    BA_COLS = 12672

    with tile.TileContext(nc) as tc:
        with (
            tc.tile_pool(name="static", bufs=1) as static,
            tc.tile_pool(name="babsp", bufs=3) as babsp,
            tc.tile_pool(name="dexpp", bufs=2) as dexpp,
            tc.tile_pool(name="et0p", bufs=8) as et0p,
            tc.tile_pool(name="et1p", bufs=2) as et1p,
            tc.tile_pool(name="dramp", bufs=1, space="DRAM") as dramp,
        ):
            # ---- stage 1: load inputs, M = x @ T_local ---------------------
            xt_sb = static.tile([P, CC * B], bf16, tag="xt")
            t_sb = static.tile([P, CC * N_LOC], bf16, tag="t")
            for cc in range(CC):
                nc.sync.dma_start(out=xt_sb[:, cc * B:(cc + 1) * B],
                                  in_=xT_d[cc * P:(cc + 1) * P, :])
                nc.sync.dma_start(out=t_sb[:, cc * N_LOC:(cc + 1) * N_LOC],
                                  in_=T_d[cc * P:(cc + 1) * P, :])

            ident = static.tile([P, P], bf16, tag="ident")
            make_identity(nc, ident[:, :])
            identf = static.tile([JBANK, JBANK], f32, tag="identf")
            make_identity(nc, identf[:, :])
            ones_col = static.tile([P, 1], f32, tag="ones_col")
            nc.vector.memset(ones_col[:, :], 1.0)

            negm_d = dramp.tile([B, N_LOC], bf16, tag="negm_d")
            m_bf = []
            with tc.tile_pool(name="mmp", bufs=2, space="PSUM") as mmp:
                for it in range(ITILES):
                    pm = mmp.tile([P, N_LOC], f32, tag="pm")
                    for cc in range(CC):
                        nc.tensor.matmul(
                            pm[:, :],
                            lhsT=xt_sb[:, cc * B + it * P: cc * B + it * P + P],
                            rhs=t_sb[:, cc * N_LOC:(cc + 1) * N_LOC],
                            start=(cc == 0), stop=(cc == CC - 1),
                        )
                    mb = static.tile([P, N_LOC], bf16, tag=f"mbf{it}",
                                     name=f"mbf{it}")
                    nc.scalar.copy(mb[:, :], pm[:, :])
                    m_bf.append(mb)
                    ng = static.tile([P, N_LOC], bf16, tag=f"neg{it}",
                                     name=f"neg{it}")
                    nc.vector.tensor_scalar(out=ng[:, :], in0=mb[:, :],
                                            scalar1=-1.0, scalar2=None,
                                            op0=Alu.mult)
                    nc.sync.dma_start(out=negm_d[it * P:(it + 1) * P, :],
                                      in_=ng[:, :])

            # ---- stage 2: lhsT tiles  [M_o^T (50 rows); ones] --------------
            # lhsT tiles [M_o^T (50 rows); ones] — ones row arrives by
            # DMA from rp row 51 (partition 50 is not engine-alignable)
            lhs = []
            with tc.tile_pool(name="tpp", bufs=2, space="PSUM") as tpp:
                for o in range(O_LOC):
                    lt = static.tile([K + 1, B], bf16, tag=f"lhs{o}",
                                     name=f"lhs{o}")
                    for it in range(ITILES):
                        tp = tpp.tile([K, P], bf16, tag="tp")
                        nc.tensor.transpose(
                            tp[:, :], m_bf[it][:, o * K64: o * K64 + K],
                            ident[:, :])
                        nc.scalar.copy(lt[0:K, it * P:(it + 1) * P], tp[:, :])
                    nc.sync.dma_start(out=lt[K:K + 1, 0:B],
                                      in_=rp_d[K + 1:K + 2, 0:B])
                    lhs.append(lt)

            # ---- stage 3: rhs tiles [I50 tiled over j ; -M_o row] ----------
            # rows 0..49 (tiled identity) and the lhs ones-row ship
            # pre-built from the host via rp; the -M_o row lands at
            # partition 50 by DMA (no engine partition-alignment limits)
            rhs_t = []
            for h in range(2):
                rt = static.tile([K + 1, JK], bf16, tag=f"rhs{h}",
                                 name=f"rhs{h}")
                nc.sync.dma_start(out=rt[:, :], in_=rp_d[0:K + 1, :])
                rhs_t.append(rt)

            # ---- stage 4: per (o, itile): diffs -> |.| -> k-sum -> exp -----
            feat_sb = [static.tile([P, O_LOC], f32, tag=f"feat{it}",
                                   name=f"feat{it}")
                       for it in range(ITILES)]
            et0_tiles = []
            group_idx = 0
            stage4 = tc.tile_pool(name="chp", bufs=2, space="PSUM")
            chp = stage4.__enter__()
            for o in range(O_LOC):
                rt = rhs_t[o % 2]
                nc.sync.dma_start(
                    out=rt[K:K + 1, :].rearrange("p (j k) -> p j k", k=K),
                    in_=negm_d[:, o * K64: o * K64 + K].rearrange(
                        "(o j) k -> o j k", o=1))
                for it in range(ITILES):
                    g_lo = 0 if it == 0 else NGROUP // 2
                    nj = (NGROUP - g_lo) * GJ
                    dexp = dexpp.tile([P, B], f32, tag="dexp")
                    for g in range(g_lo, NGROUP):
                        direct = group_idx % DIRECT_EVERY == 0
                        gsl = dexp[:, (g - g_lo) * GJ:(g - g_lo + 1) * GJ]
                        if not direct:
                            ba = babsp.tile([P, BA_COLS], bf16, tag="ba")
                        for cc in range(CPG):
                            c = g * CPG + cc
                            ch = chp.tile([P, CH], f32, tag="ch")
                            for q in range(QB):
                                col = (c * JCHUNK + q * JBANK) * K
                                nc.tensor.matmul(
                                    ch[:, q * 512: q * 512 + JBANK * K],
                                    lhsT=lhs[o][:, it * P:(it + 1) * P],
                                    rhs=rt[:, col: col + JBANK * K],
                                    start=True, stop=True)
                            # PSUM chunk viewed [p, q(4), j(8), k(50)]
                            ch4 = ch[:, :].rearrange(
                                "p (q r) -> p q r", q=QB)[
                                :, :, 0:JBANK * K].rearrange(
                                "p q (j k) -> p q j k", k=K)
                            if direct:
                                # DVE: fused |.| + k-reduce from PSUM
                                nc.vector.tensor_reduce(
                                    out=gsl[:, cc * JCHUNK:
                                            (cc + 1) * JCHUNK].rearrange(
                                        "p (q j) -> p q j", q=QB),
                                    in_=ch4,
                                    axis=mybir.AxisListType.X, op=Alu.add,
                                    apply_absolute_value=True)
                            else:
                                # ScalarE |.| cast to bf16 (dense j-major)
                                nc.scalar.activation(
                                    out=ba[:, cc * JCHUNK * K:
                                           (cc + 1) * JCHUNK * K].rearrange(
                                        "p (q j k) -> p q j k",
                                        q=QB, j=JBANK),
                                    in_=ch4, func=Act.Abs)
                        if not direct:
                            # group tree: 128 (c,q,j) groups x k, dense
                            # even-width halvings at DVE 2x; odd leftovers
                            # pair-added into 1-wide regions
                            def view(ofs, width):
                                return ba[:, ofs: ofs + GJ * width].\
                                    rearrange("p (g k) -> p g k", k=width)
                            cur, w = 0, K
                            free = GJ * K
                            singles = []
                            while w > 1:
                                hw = w // 2
                                if hw > 1 and hw % 2:
                                    hw -= 1
                                src = view(cur, w)
                                rem = w - 2 * hw
                                if rem == 1:
                                    singles.append(src[:, :, w - 1:w])
                                elif rem == 2:
                                    nc.vector.tensor_tensor(
                                        out=view(free, 1),
                                        in0=src[:, :, w - 2:w - 1],
                                        in1=src[:, :, w - 1:w],
                                        op=Alu.add)
                                    singles.append(view(free, 1))
                                    free += GJ
                                nc.vector.tensor_tensor(
                                    out=view(free, hw),
                                    in0=src[:, :, 0:hw],
                                    in1=src[:, :, hw:2 * hw],
                                    op=Alu.add)
                                cur = free
                                free += hw * GJ
                                w = hw
                            gsl3 = gsl.rearrange("p (g k) -> p g k", k=1)
                            for si, sv in enumerate(singles):
                                last = si == len(singles) - 1
                                dst = gsl3 if last else view(free, 1)
                                nc.vector.tensor_tensor(
                                    out=dst, in0=view(cur, 1), in1=sv,
                                    op=Alu.add)
                                cur = free
                                free += GJ
                            if not singles:
                                nc.vector.tensor_copy(out=gsl3,
                                                      in_=view(cur, 1))
                        group_idx += 1
                    if it == 0:
                        et = et0p.tile([P, B], f32, tag="et0",
                                       name=f"et0_{o}")
                        et0_tiles.append(et)
                    else:
                        et = et1p.tile([P, B // 2], f32, tag="et1")
                    nc.scalar.activation(out=et[:, :], in_=dexp[:, 0:nj],
                                         func=Act.Exp, scale=-1.0)
                    nc.vector.tensor_reduce(
                        out=feat_sb[it][:, o:o + 1], in_=et[:, :],
                        axis=mybir.AxisListType.X, op=Alu.add)
            stage4.__exit__(None, None, None)

            # ---- stage 5: mirrored contribution for itile 1 ----------------
            # colsum_o[j] = sum_{i in it0} exp(-D[i, j]) for j in [128, 256)
            cs_sb = static.tile([JBANK, P], f32, tag="cs_sb")
            with tc.tile_pool(name="csp", bufs=2, space="PSUM") as csp:
                for o in range(O_LOC):
                    cs = csp.tile([1, P], f32, tag="cs")
                    nc.tensor.matmul(cs[:, :], lhsT=ones_col[:, :],
                                     rhs=et0_tiles[o][:, P:B],
                                     start=True, stop=True)
                    cs_row = babsp.tile([1, P], f32, tag="cs_row")
                    nc.scalar.copy(cs_row[:, :], cs[:, :])
                    nc.sync.dma_start(out=cs_sb[o:o + 1, :], in_=cs_row[:, :])
                ct = csp.tile([P, JBANK], f32, tag="ct")
                nc.tensor.transpose(ct[:, :], cs_sb[:, :], identf[:, :])
                nc.vector.tensor_tensor(out=feat_sb[1][:, :],
                                        in0=feat_sb[1][:, :],
                                        in1=ct[:, :], op=Alu.add)

            for it in range(ITILES):
                nc.vector.tensor_scalar(
                    out=feat_sb[it][:, :], in0=feat_sb[it][:, :],
                    scalar1=1.0, scalar2=None, op0=Alu.subtract)
                nc.sync.dma_start(out=feat_d[it * P:(it + 1) * P, :],
                                  in_=feat_sb[it][:, :])

    nc.compile()
    return nc


def _get_program():
    if "nc" not in _cache:
        _cache["nc"] = _build_program()
    return _cache["nc"]


def prepare_in_maps(x, T):
    """Host-side sharding: transpose/cast x, slice + K-pad T per core."""
    bf = ml_dtypes.bfloat16
    xT = np.ascontiguousarray(np.asarray(x, dtype=np.float32).T).astype(bf)
    Tf = np.asarray(T, dtype=np.float32)
    in_maps = []
    rp = np.zeros((K + 2, K * B), dtype=bf)
    kk = np.arange(K)
    for j in range(B):
        rp[kk, j * K + kk] = 1.0
    rp[K + 1, :] = 1.0
    for c in range(N_CORES):
        Tl = np.zeros((IN_FEATURES, N_LOC), dtype=bf)
        for o in range(O_LOC):
            g = (c * O_LOC + o) * K
            Tl[:, o * K64: o * K64 + K] = Tf[:, g: g + K].astype(bf)
        in_maps.append({"xT": xT, "Tl": Tl, "rp": rp})
    return in_maps


def run_cores(in_maps, trace=False, tmpdir=None):
    from concourse import bass_utils
    nc = _get_program()
    return bass_utils.run_bass_kernel_spmd(
        nc, in_maps, core_ids=list(range(N_CORES)), trace=trace, tmpdir=tmpdir)


def kernel(x, T):
    x = np.asarray(x, dtype=np.float32)
    res = run_cores(prepare_in_maps(x, T))
    feat = np.concatenate(
        [res.results[c]["feat"].astype(np.float32) for c in range(N_CORES)],
        axis=1)
    return np.concatenate([x, feat], axis=1)


# revision 29
# speedup vs baseline: 1.0112x; 1.0112x over previous
"""Trainium2 Bass kernel for MinibatchDiscrimination.

Reference computation (B=256, IN=1024, O=64, K=50):
    M = (x @ T).reshape(B, O, K)
    l1[i,j,o] = sum_k |M[i,o,k] - M[j,o,k]|
    out = concat([x, sum_j exp(-l1) - 1], axis=1)          # [B, IN + O]

Sharding: the O (out_features) dimension is split across the 8 NeuronCores
(8 features per core); x is replicated. Each core computes its [256, 8]
feature block; the host gathers the blocks and concatenates with x.

Per-core pipeline:
  1. PE matmul: M[256, 512] = xT.T @ T_local (bf16 in, f32 PSUM; K padded to
     64 only for this GEMM), cast to bf16 — the canonical value used on BOTH
     sides of the pairwise subtraction, so the diagonal distance is exactly
     zero. -M is staged to DRAM.
  2. All-pairs signed differences are generated by the PE with an affine
     matmul: diff[i, (j,k)] = sum_p lhsT[p,i] * rhs[p,(j,k)] with
     lhsT = [M_o^T (50 k-rows); ones] and rhs = [I50 tiled over j; -M_o row].
     Chunks of 32 j land in PSUM as [128, 4x512] f32 (400 of each 512-col
     bank used: 8 j x 50 k).
  3. Exploiting D[i,j] = D[j,i], the itile-1 blocks only compute j in
     [128,256); the mirrored contribution comes from PE column-sums of the
     itile-0 exp tiles at the end.
  4. Each PSUM chunk takes one of two abs+k-reduce paths (balancing DVE and
     ScalarE): (a) DVE tensor_reduce(add, apply_absolute_value) straight
     from PSUM, or (b) ScalarE Abs-cast to bf16 SBUF (written k-major) +
     dense DVE binary-tree tensor_tensor adds at 2x.
  5. ScalarE exp(-l1) (scale=-1), DVE reduce over j, -1.0, DMA out.
"""

import numpy as np
import ml_dtypes

B = 256
IN_FEATURES = 1024
O_TOTAL = 64
K = 50
K64 = 64
N_CORES = 8
O_LOC = O_TOTAL // N_CORES          # 8 features per core
N_LOC = O_LOC * K                   # 400 M columns per core
P = 128                             # partitions
ITILES = B // P                     # 2 row tiles
CC = IN_FEATURES // P               # 8 contraction chunks
JCHUNK = 32                         # j's per PSUM chunk
JBANK = 8                           # j's per PSUM bank (8*50 = 400 of 512)
QB = JCHUNK // JBANK                # banks per chunk = 4
NCHUNK = B // JCHUNK                # 8 chunks per full block
CPG = 4                             # chunks per tree group
NGROUP = NCHUNK // CPG              # 2 groups per full block
GJ = CPG * JCHUNK                   # 128 j's (= (c,q,j) groups) per tree
DIRECT_EVERY = 5                    # every Nth GROUP takes the DVE-direct path

_cache = {}


def _build_program():
    import concourse.mybir as mybir
    from concourse import bacc, tile
    from concourse.masks import make_identity

    f32 = mybir.dt.float32
    bf16 = mybir.dt.bfloat16
    Alu = mybir.AluOpType
    Act = mybir.ActivationFunctionType

    nc = bacc.Bacc("TRN2", target_bir_lowering=False, debug=False,
                   enable_asserts=False)

    xT_d = nc.dram_tensor("xT", [IN_FEATURES, B], bf16, kind="ExternalInput").ap()
    T_d = nc.dram_tensor("Tl", [IN_FEATURES, N_LOC], bf16, kind="ExternalInput").ap()
    rp_d = nc.dram_tensor("rp", [K + 2, K * B], bf16,
                          kind="ExternalInput").ap()
    feat_d = nc.dram_tensor("feat", [B, O_LOC], f32, kind="ExternalOutput").ap()

    JK = K * B                      # 12800 diff columns per full block
    CH = QB * 512                   # 2048 PSUM elements per chunk (1600 used)
    # ba scratch: 4-chunk level-0 (128 groups x 50) + tree level regions
    BA_COLS = 12672

    with tile.TileContext(nc) as tc:
        with (
            tc.tile_pool(name="static", bufs=1) as static,
            tc.tile_pool(name="babsp", bufs=3) as babsp,
            tc.tile_pool(name="dexpp", bufs=2) as dexpp,
            tc.tile_pool(name="et0p", bufs=8) as et0p,
            tc.tile_pool(name="et1p", bufs=2) as et1p,
            tc.tile_pool(name="dramp", bufs=1, space="DRAM") as dramp,
        ):
            # ---- stage 1: load inputs, M = x @ T_local ---------------------
            xt_sb = static.tile([P, CC * B], bf16, tag="xt")
            t_sb = static.tile([P, CC * N_LOC], bf16, tag="t")
            for cc in range(CC):
                nc.sync.dma_start(out=xt_sb[:, cc * B:(cc + 1) * B],
                                  in_=xT_d[cc * P:(cc + 1) * P, :])
                nc.scalar.dma_start(out=t_sb[:, cc * N_LOC:(cc + 1) * N_LOC],
                                    in_=T_d[cc * P:(cc + 1) * P, :])

            ident = static.tile([P, P], bf16, tag="ident")
            make_identity(nc, ident[:, :])
            identf = static.tile([JBANK, JBANK], f32, tag="identf")
            make_identity(nc, identf[:, :])
            ones_col = static.tile([P, 1], f32, tag="ones_col")
            nc.vector.memset(ones_col[:, :], 1.0)

            # -M staged to DRAM as one flat j-major row per o, so the
            # per-o rhs row refresh is a single contiguous 25.6KB packet
            negm_d = dramp.tile([O_LOC, K * B], bf16, tag="negm_d")
            m_bf = []
            with tc.tile_pool(name="mmp", bufs=2, space="PSUM") as mmp:
                for it in range(ITILES):
                    pm = mmp.tile([P, N_LOC], f32, tag="pm")
                    for cc in range(CC):
                        nc.tensor.matmul(
                            pm[:, :],
                            lhsT=xt_sb[:, cc * B + it * P: cc * B + it * P + P],
                            rhs=t_sb[:, cc * N_LOC:(cc + 1) * N_LOC],
                            start=(cc == 0), stop=(cc == CC - 1),
                        )
                    mb = static.tile([P, N_LOC], bf16, tag=f"mbf{it}",
                                     name=f"mbf{it}")
                    nc.scalar.copy(mb[:, :], pm[:, :])
                    m_bf.append(mb)
                    ng = static.tile([P, N_LOC], bf16, tag=f"neg{it}",
                                     name=f"neg{it}")
                    nc.vector.tensor_scalar(out=ng[:, :], in0=mb[:, :],
                                            scalar1=-1.0, scalar2=None,
                                            op0=Alu.mult)
                    half = K * P
                    for o in range(O_LOC):
                        nc.sync.dma_start(
                            out=negm_d[o:o + 1,
                                       it * half:(it + 1) * half],
                            in_=ng[:, o * K:(o + 1) * K])

            # ---- stage 2: lhsT tiles  [M_o^T (50 rows); ones] --------------
            # lhsT tiles [M_o^T (50 rows); ones] — ones row arrives by
            # DMA from rp row 51 (partition 50 is not engine-alignable)
            lhs = []
            with tc.tile_pool(name="tpp", bufs=2, space="PSUM") as tpp:
                for o in range(O_LOC):
                    lt = static.tile([K + 1, B], bf16, tag=f"lhs{o}",
                                     name=f"lhs{o}")
                    for it in range(ITILES):
                        tp = tpp.tile([K, P], bf16, tag="tp")
                        nc.tensor.transpose(
                            tp[:, :], m_bf[it][:, o * K: o * K + K],
                            ident[:, :])
                        nc.scalar.copy(lt[0:K, it * P:(it + 1) * P], tp[:, :])
                    nc.sync.dma_start(out=lt[K:K + 1, 0:B],
                                      in_=rp_d[K + 1:K + 2, 0:B])
                    lhs.append(lt)

            # ---- stage 3: rhs tiles [I50 tiled over j ; -M_o row] ----------
            # rows 0..49 (tiled identity) and the lhs ones-row ship
            # pre-built from the host via rp; the -M_o row lands at
            # partition 50 by DMA (no engine partition-alignment limits)
            rhs_t = []
            for h in range(2):
                rt = static.tile([K + 1, JK], bf16, tag=f"rhs{h}",
                                 name=f"rhs{h}")
                nc.scalar.dma_start(out=rt[:, :], in_=rp_d[0:K + 1, :])
                rhs_t.append(rt)

            # ---- stage 4: per (o, itile): diffs -> |.| -> k-sum -> exp -----
            feat_sb = [static.tile([P, O_LOC], f32, tag=f"feat{it}",
                                   name=f"feat{it}")
                       for it in range(ITILES)]
            et0_tiles = []
            group_idx = 0
            stage4 = tc.tile_pool(name="chp", bufs=2, space="PSUM")
            chp = stage4.__enter__()
            for o in range(O_LOC):
                rt = rhs_t[o % 2]
                nc.sync.dma_start(out=rt[K:K + 1, :],
                                  in_=negm_d[o:o + 1, :])
                for it in range(ITILES):
                    g_lo = 0 if it == 0 else NGROUP // 2
                    nj = (NGROUP - g_lo) * GJ
                    dexp = dexpp.tile([P, B], f32, tag="dexp")
                    for g in range(g_lo, NGROUP):
                        direct = group_idx % DIRECT_EVERY == 0
                        gsl = dexp[:, (g - g_lo) * GJ:(g - g_lo + 1) * GJ]
                        if not direct:
                            ba = babsp.tile([P, BA_COLS], bf16, tag="ba")
                        for cc in range(CPG):
                            c = g * CPG + cc
                            ch = chp.tile([P, CH], f32, tag="ch")
                            for q in range(QB):
                                col = (c * JCHUNK + q * JBANK) * K
                                nc.tensor.matmul(
                                    ch[:, q * 512: q * 512 + JBANK * K],
                                    lhsT=lhs[o][:, it * P:(it + 1) * P],
                                    rhs=rt[:, col: col + JBANK * K],
                                    start=True, stop=True)
                            # PSUM chunk viewed [p, q(4), j(8), k(50)]
                            ch4 = ch[:, :].rearrange(
                                "p (q r) -> p q r", q=QB)[
                                :, :, 0:JBANK * K].rearrange(
                                "p q (j k) -> p q j k", k=K)
                            if direct:
                                # DVE: fused |.| + k-reduce from PSUM
                                nc.vector.tensor_reduce(
                                    out=gsl[:, cc * JCHUNK:
                                            (cc + 1) * JCHUNK].rearrange(
                                        "p (q j) -> p q j", q=QB),
                                    in_=ch4,
                                    axis=mybir.AxisListType.X, op=Alu.add,
                                    apply_absolute_value=True)
                            else:
                                # ScalarE |.| cast to bf16 (dense j-major)
                                nc.scalar.activation(
                                    out=ba[:, cc * JCHUNK * K:
                                           (cc + 1) * JCHUNK * K].rearrange(
                                        "p (q j k) -> p q j k",
                                        q=QB, j=JBANK),
                                    in_=ch4, func=Act.Abs)
                        if not direct:
                            # group tree: 128 (c,q,j) groups x k, dense
                            # even-width halvings at DVE 2x; odd leftovers
                            # pair-added into 1-wide regions
                            def view(ofs, width):
                                return ba[:, ofs: ofs + GJ * width].\
                                    rearrange("p (g k) -> p g k", k=width)
                            cur, w = 0, K
                            free = GJ * K
                            singles = []
                            while w > 1:
                                hw = w // 2
                                if hw > 1 and hw % 2:
                                    hw -= 1
                                src = view(cur, w)
                                rem = w - 2 * hw
                                if rem == 1:
                                    singles.append(src[:, :, w - 1:w])
                                elif rem == 2:
                                    nc.vector.tensor_tensor(
                                        out=view(free, 1),
                                        in0=src[:, :, w - 2:w - 1],
                                        in1=src[:, :, w - 1:w],
                                        op=Alu.add)
                                    singles.append(view(free, 1))
                                    free += GJ
                                nc.vector.tensor_tensor(
                                    out=view(free, hw),
                                    in0=src[:, :, 0:hw],
                                    in1=src[:, :, hw:2 * hw],
                                    op=Alu.add)
                                cur = free
                                free += hw * GJ
                                w = hw
                            gsl3 = gsl.rearrange("p (g k) -> p g k", k=1)
                            for si, sv in enumerate(singles):
                                last = si == len(singles) - 1
                                dst = gsl3 if last else view(free, 1)
                                nc.vector.tensor_tensor(
                                    out=dst, in0=view(cur, 1), in1=sv,
                                    op=Alu.add)
                                cur = free
                                free += GJ
                            if not singles:
                                nc.vector.tensor_copy(out=gsl3,
                                                      in_=view(cur, 1))
                        group_idx += 1
                    if it == 0:
                        et = et0p.tile([P, B], f32, tag="et0",
                                       name=f"et0_{o}")
                        et0_tiles.append(et)
                    else:
                        et = et1p.tile([P, B // 2], f32, tag="et1")
                    nc.scalar.activation(out=et[:, :], in_=dexp[:, 0:nj],
                                         func=Act.Exp, scale=-1.0)
                    nc.vector.tensor_reduce(
                        out=feat_sb[it][:, o:o + 1], in_=et[:, :],
                        axis=mybir.AxisListType.X, op=Alu.add)
            stage4.__exit__(None, None, None)

            # ---- stage 5: mirrored contribution for itile 1 ----------------
            # colsum_o[j] = sum_{i in it0} exp(-D[i, j]) for j in [128, 256)
            cs_sb = static.tile([JBANK, P], f32, tag="cs_sb")
            with tc.tile_pool(name="csp", bufs=2, space="PSUM") as csp:
                for o in range(O_LOC):
                    cs = csp.tile([1, P], f32, tag="cs")
                    nc.tensor.matmul(cs[:, :], lhsT=ones_col[:, :],
                                     rhs=et0_tiles[o][:, P:B],
                                     start=True, stop=True)
                    cs_row = babsp.tile([1, P], f32, tag="cs_row")
                    nc.scalar.copy(cs_row[:, :], cs[:, :])
                    nc.sync.dma_start(out=cs_sb[o:o + 1, :], in_=cs_row[:, :])
                ct = csp.tile([P, JBANK], f32, tag="ct")
                nc.tensor.transpose(ct[:, :], cs_sb[:, :], identf[:, :])
                nc.vector.tensor_tensor(out=feat_sb[1][:, :],
                                        in0=feat_sb[1][:, :],
                                        in1=ct[:, :], op=Alu.add)

            for it in range(ITILES):
                nc.vector.tensor_scalar(
                    out=feat_sb[it][:, :], in0=feat_sb[it][:, :],
                    scalar1=1.0, scalar2=None, op0=Alu.subtract)
                nc.sync.dma_start(out=feat_d[it * P:(it + 1) * P, :],
                                  in_=feat_sb[it][:, :])

    nc.compile()
    return nc


def _get_program():
    if "nc" not in _cache:
        _cache["nc"] = _build_program()
    return _cache["nc"]


def prepare_in_maps(x, T):
    """Host-side sharding: transpose/cast x, slice + K-pad T per core."""
    bf = ml_dtypes.bfloat16
    xT = np.ascontiguousarray(np.asarray(x, dtype=np.float32).T).astype(bf)
    Tf = np.asarray(T, dtype=np.float32)
    in_maps = []
    rp = np.zeros((K + 2, K * B), dtype=bf)
    kk = np.arange(K)
    for j in range(B):
        rp[kk, j * K + kk] = 1.0
    rp[K + 1, :] = 1.0
    for c in range(N_CORES):
        Tl = np.ascontiguousarray(
            Tf[:, c * N_LOC:(c + 1) * N_LOC]).astype(bf)
        in_maps.append({"xT": xT, "Tl": Tl, "rp": rp})
    return in_maps


def run_cores(in_maps, trace=False, tmpdir=None):
    from concourse import bass_utils
    nc = _get_program()
    return bass_utils.run_bass_kernel_spmd(
        nc, in_maps, core_ids=list(range(N_CORES)), trace=trace, tmpdir=tmpdir)


def kernel(x, T):
    x = np.asarray(x, dtype=np.float32)
    res = run_cores(prepare_in_maps(x, T))
    feat = np.concatenate(
        [res.results[c]["feat"].astype(np.float32) for c in range(N_CORES)],
        axis=1)
    return np.concatenate([x, feat], axis=1)


# revision 31
# speedup vs baseline: 1.1018x; 1.0897x over previous
"""Trainium2 Bass kernel for MinibatchDiscrimination.

Reference computation (B=256, IN=1024, O=64, K=50):
    M = (x @ T).reshape(B, O, K)
    l1[i,j,o] = sum_k |M[i,o,k] - M[j,o,k]|
    out = concat([x, sum_j exp(-l1) - 1], axis=1)          # [B, IN + O]

Sharding: the O (out_features) dimension is split across the 8 NeuronCores
(8 features per core); x is replicated. Each core computes its [256, 8]
feature block; the host gathers the blocks and concatenates with x.

Per-core pipeline:
  1. PE matmul: M[256, 512] = xT.T @ T_local (bf16 in, f32 PSUM; K padded to
     64 only for this GEMM), cast to bf16 — the canonical value used on BOTH
     sides of the pairwise subtraction, so the diagonal distance is exactly
     zero. -M is staged to DRAM.
  2. All-pairs signed differences are generated by the PE with an affine
     matmul: diff[i, (j,k)] = sum_p lhsT[p,i] * rhs[p,(j,k)] with
     lhsT = [M_o^T (50 k-rows); ones] and rhs = [I50 tiled over j; -M_o row].
     Chunks of 32 j land in PSUM as [128, 4x512] f32 (400 of each 512-col
     bank used: 8 j x 50 k).
  3. Exploiting D[i,j] = D[j,i], the itile-1 blocks only compute j in
     [128,256); the mirrored contribution comes from PE column-sums of the
     itile-0 exp tiles at the end.
  4. Each PSUM chunk takes one of two abs+k-reduce paths (balancing DVE and
     ScalarE): (a) DVE tensor_reduce(add, apply_absolute_value) straight
     from PSUM, or (b) ScalarE Abs-cast to bf16 SBUF (written k-major) +
     dense DVE binary-tree tensor_tensor adds at 2x.
  5. ScalarE exp(-l1) (scale=-1), DVE reduce over j, -1.0, DMA out.
"""

import numpy as np
import ml_dtypes

B = 256
IN_FEATURES = 1024
O_TOTAL = 64
K = 50
K64 = 64
N_CORES = 8
O_LOC = O_TOTAL // N_CORES          # 8 features per core
N_LOC = O_LOC * K                   # 400 M columns per core
P = 128                             # partitions
ITILES = B // P                     # 2 row tiles
CC = IN_FEATURES // P               # 8 contraction chunks
JCHUNK = 32                         # j's per PSUM chunk
JBANK = 8                           # j's per PSUM bank (8*50 = 400 of 512)
QB = JCHUNK // JBANK                # banks per chunk = 4
NCHUNK = B // JCHUNK                # 8 chunks per full block
CPG = 4                             # chunks per tree group
NGROUP = NCHUNK // CPG              # 2 groups per full block
GJ = CPG * JCHUNK                   # 128 j's (= (c,q,j) groups) per tree
DIRECT_EVERY = 5                    # every Nth GROUP takes the DVE-direct path

_cache = {}


def _build_program():
    import concourse.mybir as mybir
    from concourse import bacc, tile
    from concourse.masks import make_identity

    f32 = mybir.dt.float32
    bf16 = mybir.dt.bfloat16
    fp8 = mybir.dt.float8e4
    Alu = mybir.AluOpType
    Act = mybir.ActivationFunctionType

    nc = bacc.Bacc("TRN2", target_bir_lowering=False, debug=False,
                   enable_asserts=False)

    xT_d = nc.dram_tensor("xT", [IN_FEATURES, B], bf16, kind="ExternalInput").ap()
    T_d = nc.dram_tensor("Tl", [IN_FEATURES, N_LOC], bf16, kind="ExternalInput").ap()
    rp_d = nc.dram_tensor("rp", [K + 2, K * B], fp8,
                          kind="ExternalInput").ap()
    feat_d = nc.dram_tensor("feat", [B, O_LOC], f32, kind="ExternalOutput").ap()

    JK = K * B                      # 12800 diff columns per full block
    CH = QB * 512                   # 2048 PSUM elements per chunk (1600 used)
    # ba scratch: 4-chunk level-0 (128 groups x 50) + tree level regions
    BA_COLS = 12672

    with tile.TileContext(nc) as tc:
        with (
            tc.tile_pool(name="static", bufs=1) as static,
            tc.tile_pool(name="babsp", bufs=3) as babsp,
            tc.tile_pool(name="dexpp", bufs=2) as dexpp,
            tc.tile_pool(name="et0p", bufs=8) as et0p,
            tc.tile_pool(name="et1p", bufs=2) as et1p,
            tc.tile_pool(name="dramp", bufs=1, space="DRAM") as dramp,
        ):
            # ---- rhs I-part loads first: they gate the pairwise stage ----
            rhs_t = []
            for h in range(2):
                rt = static.tile([K + 1, JK], fp8, tag=f"rhs{h}",
                                 name=f"rhs{h}")
                nc.sync.dma_start(out=rt[:, 0:JK // 2],
                                  in_=rp_d[0:K + 1, 0:JK // 2])
                nc.scalar.dma_start(out=rt[:, JK // 2:],
                                    in_=rp_d[0:K + 1, JK // 2:])
                rhs_t.append(rt)

            # ---- stage 1: load inputs, M = x @ T_local ---------------------
            xt_sb = static.tile([P, CC * B], bf16, tag="xt")
            t_sb = static.tile([P, CC * N_LOC], bf16, tag="t")
            for cc in range(CC):
                nc.sync.dma_start(out=xt_sb[:, cc * B:(cc + 1) * B],
                                  in_=xT_d[cc * P:(cc + 1) * P, :])
                nc.scalar.dma_start(out=t_sb[:, cc * N_LOC:(cc + 1) * N_LOC],
                                    in_=T_d[cc * P:(cc + 1) * P, :])

            ident = static.tile([P, P], bf16, tag="ident")
            make_identity(nc, ident[:, :])
            identf = static.tile([JBANK, JBANK], f32, tag="identf")
            make_identity(nc, identf[:, :])
            ones_col = static.tile([P, 1], f32, tag="ones_col")
            nc.vector.memset(ones_col[:, :], 1.0)

            # -M staged to DRAM as one flat j-major row per o, so the
            # per-o rhs row refresh is a single contiguous 25.6KB packet
            negm_d = dramp.tile([O_LOC, K * B], fp8, tag="negm_d")
            m_bf = []
            m_bb = []
            with tc.tile_pool(name="mmp", bufs=2, space="PSUM") as mmp:
                for it in range(ITILES):
                    pm = mmp.tile([P, N_LOC], f32, tag="pm")
                    for cc in range(CC):
                        nc.tensor.matmul(
                            pm[:, :],
                            lhsT=xt_sb[:, cc * B + it * P: cc * B + it * P + P],
                            rhs=t_sb[:, cc * N_LOC:(cc + 1) * N_LOC],
                            start=(cc == 0), stop=(cc == CC - 1),
                        )
                    mb = static.tile([P, N_LOC], fp8, tag=f"mbf{it}",
                                     name=f"mbf{it}")
                    nc.scalar.copy(mb[:, :], pm[:, :])
                    m_bf.append(mb)
                    mbb = static.tile([P, N_LOC], bf16, tag=f"mbb{it}",
                                      name=f"mbb{it}")
                    nc.scalar.copy(mbb[:, :], mb[:, :])
                    m_bb.append(mbb)
                    ng = static.tile([P, N_LOC], fp8, tag=f"neg{it}",
                                     name=f"neg{it}")
                    nc.vector.tensor_scalar(out=ng[:, :], in0=mb[:, :],
                                            scalar1=-1.0, scalar2=None,
                                            op0=Alu.mult)
                    half = K * P
                    for o in range(O_LOC):
                        nc.sync.dma_start(
                            out=negm_d[o:o + 1,
                                       it * half:(it + 1) * half],
                            in_=ng[:, o * K:(o + 1) * K])

            # ---- stage 2: lhsT tiles  [M_o^T (50 rows); ones] --------------
            # lhsT tiles [M_o^T (50 rows); ones] — ones row arrives by
            # DMA from rp row 51 (partition 50 is not engine-alignable)
            lhs = []
            with tc.tile_pool(name="tpp", bufs=2, space="PSUM") as tpp:
                for o in range(O_LOC):
                    lt = static.tile([K + 1, B], fp8, tag=f"lhs{o}",
                                     name=f"lhs{o}")
                    for it in range(ITILES):
                        tp = tpp.tile([K, P], bf16, tag="tp")
                        nc.tensor.transpose(
                            tp[:, :], m_bb[it][:, o * K: o * K + K],
                            ident[:, :])
                        nc.scalar.copy(lt[0:K, it * P:(it + 1) * P], tp[:, :])
                    nc.sync.dma_start(out=lt[K:K + 1, 0:B],
                                      in_=rp_d[K + 1:K + 2, 0:B])
                    lhs.append(lt)

            # ---- stage 4: per (o, itile): diffs -> |.| -> k-sum -> exp -----
            feat_sb = [static.tile([P, O_LOC], f32, tag=f"feat{it}",
                                   name=f"feat{it}")
                       for it in range(ITILES)]
            et0_tiles = []
            group_idx = 0
            stage4 = tc.tile_pool(name="chp", bufs=2, space="PSUM")
            chp = stage4.__enter__()
            for o in range(O_LOC):
                rt = rhs_t[o % 2]
                nc.sync.dma_start(out=rt[K:K + 1, :],
                                  in_=negm_d[o:o + 1, :])
                for it in range(ITILES):
                    g_lo = 0 if it == 0 else NGROUP // 2
                    nj = (NGROUP - g_lo) * GJ
                    dexp = dexpp.tile([P, B], f32, tag="dexp")
                    for g in range(g_lo, NGROUP):
                        direct = group_idx % DIRECT_EVERY == 0
                        gsl = dexp[:, (g - g_lo) * GJ:(g - g_lo + 1) * GJ]
                        if not direct:
                            ba = babsp.tile([P, BA_COLS], bf16, tag="ba")
                        for cc in range(CPG):
                            c = g * CPG + cc
                            ch = chp.tile([P, CH], f32, tag="ch")
                            for q in range(QB):
                                col = (c * JCHUNK + q * JBANK) * K
                                nc.tensor.matmul(
                                    ch[:, q * 512: q * 512 + JBANK * K],
                                    lhsT=lhs[o][:, it * P:(it + 1) * P],
                                    rhs=rt[:, col: col + JBANK * K],
                                    start=True, stop=True)
                            # PSUM chunk viewed [p, q(4), j(8), k(50)]
                            ch4 = ch[:, :].rearrange(
                                "p (q r) -> p q r", q=QB)[
                                :, :, 0:JBANK * K].rearrange(
                                "p q (j k) -> p q j k", k=K)
                            if direct:
                                # DVE: fused |.| + k-reduce from PSUM
                                nc.vector.tensor_reduce(
                                    out=gsl[:, cc * JCHUNK:
                                            (cc + 1) * JCHUNK].rearrange(
                                        "p (q j) -> p q j", q=QB),
                                    in_=ch4,
                                    axis=mybir.AxisListType.X, op=Alu.add,
                                    apply_absolute_value=True)
                            else:
                                # ScalarE |.| cast to bf16 (dense j-major)
                                nc.scalar.activation(
                                    out=ba[:, cc * JCHUNK * K:
                                           (cc + 1) * JCHUNK * K].rearrange(
                                        "p (q j k) -> p q j k",
                                        q=QB, j=JBANK),
                                    in_=ch4, func=Act.Abs)
                        if not direct:
                            # group tree: 128 (c,q,j) groups x k, dense
                            # even-width halvings at DVE 2x; odd leftovers
                            # pair-added into 1-wide regions
                            def view(ofs, width):
                                return ba[:, ofs: ofs + GJ * width].\
                                    rearrange("p (g k) -> p g k", k=width)
                            cur, w = 0, K
                            free = GJ * K
                            singles = []
                            while w > 1:
                                hw = w // 2
                                if hw > 1 and hw % 2:
                                    hw -= 1
                                src = view(cur, w)
                                rem = w - 2 * hw
                                if rem == 1:
                                    singles.append(src[:, :, w - 1:w])
                                elif rem == 2:
                                    nc.vector.tensor_tensor(
                                        out=view(free, 1),
                                        in0=src[:, :, w - 2:w - 1],
                                        in1=src[:, :, w - 1:w],
                                        op=Alu.add)
                                    singles.append(view(free, 1))
                                    free += GJ
                                nc.vector.tensor_tensor(
                                    out=view(free, hw),
                                    in0=src[:, :, 0:hw],
                                    in1=src[:, :, hw:2 * hw],
                                    op=Alu.add)
                                cur = free
                                free += hw * GJ
                                w = hw
                            gsl3 = gsl.rearrange("p (g k) -> p g k", k=1)
                            for si, sv in enumerate(singles):
                                last = si == len(singles) - 1
                                dst = gsl3 if last else view(free, 1)
                                nc.vector.tensor_tensor(
                                    out=dst, in0=view(cur, 1), in1=sv,
                                    op=Alu.add)
                                cur = free
                                free += GJ
                            if not singles:
                                nc.vector.tensor_copy(out=gsl3,
                                                      in_=view(cur, 1))
                        group_idx += 1
                    if it == 0:
                        et = et0p.tile([P, B], f32, tag="et0",
                                       name=f"et0_{o}")
                        et0_tiles.append(et)
                    else:
                        et = et1p.tile([P, B // 2], f32, tag="et1")
                    nc.scalar.activation(out=et[:, :], in_=dexp[:, 0:nj],
                                         func=Act.Exp, scale=-1.0)
                    nc.vector.tensor_reduce(
                        out=feat_sb[it][:, o:o + 1], in_=et[:, :],
                        axis=mybir.AxisListType.X, op=Alu.add)
            stage4.__exit__(None, None, None)

            # ---- stage 5: mirrored contribution for itile 1 ----------------
            # colsum_o[j] = sum_{i in it0} exp(-D[i, j]) for j in [128, 256)
            cs_sb = static.tile([JBANK, P], f32, tag="cs_sb")
            with tc.tile_pool(name="csp", bufs=2, space="PSUM") as csp:
                for o in range(O_LOC):
                    cs = csp.tile([1, P], f32, tag="cs")
                    nc.tensor.matmul(cs[:, :], lhsT=ones_col[:, :],
                                     rhs=et0_tiles[o][:, P:B],
                                     start=True, stop=True)
                    cs_row = babsp.tile([1, P], f32, tag="cs_row")
                    nc.scalar.copy(cs_row[:, :], cs[:, :])
                    nc.sync.dma_start(out=cs_sb[o:o + 1, :], in_=cs_row[:, :])
                ct = csp.tile([P, JBANK], f32, tag="ct")
                nc.tensor.transpose(ct[:, :], cs_sb[:, :], identf[:, :])
                nc.vector.tensor_tensor(out=feat_sb[1][:, :],
                                        in0=feat_sb[1][:, :],
                                        in1=ct[:, :], op=Alu.add)

            for it in range(ITILES):
                nc.vector.tensor_scalar(
                    out=feat_sb[it][:, :], in0=feat_sb[it][:, :],
                    scalar1=1.0, scalar2=None, op0=Alu.subtract)
                nc.sync.dma_start(out=feat_d[it * P:(it + 1) * P, :],
                                  in_=feat_sb[it][:, :])

    nc.compile()
    return nc


def _get_program():
    if "nc" not in _cache:
        _cache["nc"] = _build_program()
    return _cache["nc"]


def prepare_in_maps(x, T):
    """Host-side sharding: transpose/cast x, slice + K-pad T per core."""
    bf = ml_dtypes.bfloat16
    xT = np.ascontiguousarray(np.asarray(x, dtype=np.float32).T).astype(bf)
    Tf = np.asarray(T, dtype=np.float32)
    in_maps = []
    rp = np.zeros((K + 2, K * B), dtype=ml_dtypes.float8_e4m3fn)
    kk = np.arange(K)
    for j in range(B):
        rp[kk, j * K + kk] = 1.0
    rp[K + 1, :] = 1.0
    for c in range(N_CORES):
        Tl = np.ascontiguousarray(
            Tf[:, c * N_LOC:(c + 1) * N_LOC]).astype(bf)
        in_maps.append({"xT": xT, "Tl": Tl, "rp": rp})
    return in_maps


def run_cores(in_maps, trace=False, tmpdir=None):
    from concourse import bass_utils
    nc = _get_program()
    return bass_utils.run_bass_kernel_spmd(
        nc, in_maps, core_ids=list(range(N_CORES)), trace=trace, tmpdir=tmpdir)


def kernel(x, T):
    x = np.asarray(x, dtype=np.float32)
    res = run_cores(prepare_in_maps(x, T))
    feat = np.concatenate(
        [res.results[c]["feat"].astype(np.float32) for c in range(N_CORES)],
        axis=1)
    return np.concatenate([x, feat], axis=1)


# revision 33
# speedup vs baseline: 1.1131x; 1.0102x over previous
"""Trainium2 Bass kernel for MinibatchDiscrimination.

Reference computation (B=256, IN=1024, O=64, K=50):
    M = (x @ T).reshape(B, O, K)
    l1[i,j,o] = sum_k |M[i,o,k] - M[j,o,k]|
    out = concat([x, sum_j exp(-l1) - 1], axis=1)          # [B, IN + O]

Sharding: the O (out_features) dimension is split across the 8 NeuronCores
(8 features per core); x is replicated. Each core computes its [256, 8]
feature block; the host gathers the blocks and concatenates with x.

Per-core pipeline:
  1. PE matmul: M[256, 512] = xT.T @ T_local (bf16 in, f32 PSUM; K padded to
     64 only for this GEMM), cast to bf16 — the canonical value used on BOTH
     sides of the pairwise subtraction, so the diagonal distance is exactly
     zero. -M is staged to DRAM.
  2. All-pairs signed differences are generated by the PE with an affine
     matmul: diff[i, (j,k)] = sum_p lhsT[p,i] * rhs[p,(j,k)] with
     lhsT = [M_o^T (50 k-rows); ones] and rhs = [I50 tiled over j; -M_o row].
     Chunks of 32 j land in PSUM as [128, 4x512] f32 (400 of each 512-col
     bank used: 8 j x 50 k).
  3. Exploiting D[i,j] = D[j,i], the itile-1 blocks only compute j in
     [128,256); the mirrored contribution comes from PE column-sums of the
     itile-0 exp tiles at the end.
  4. Each PSUM chunk takes one of two abs+k-reduce paths (balancing DVE and
     ScalarE): (a) DVE tensor_reduce(add, apply_absolute_value) straight
     from PSUM, or (b) ScalarE Abs-cast to bf16 SBUF (written k-major) +
     dense DVE binary-tree tensor_tensor adds at 2x.
  5. ScalarE exp(-l1) (scale=-1), DVE reduce over j, -1.0, DMA out.
"""

import numpy as np
import ml_dtypes

B = 256
IN_FEATURES = 1024
O_TOTAL = 64
K = 50
K64 = 64
N_CORES = 8
O_LOC = O_TOTAL // N_CORES          # 8 features per core
N_LOC = O_LOC * K                   # 400 M columns per core
P = 128                             # partitions
ITILES = B // P                     # 2 row tiles
CC = IN_FEATURES // P               # 8 contraction chunks
JCHUNK = 32                         # j's per PSUM chunk
JBANK = 8                           # j's per PSUM bank (8*50 = 400 of 512)
QB = JCHUNK // JBANK                # banks per chunk = 4
NCHUNK = B // JCHUNK                # 8 chunks per full block
CPG = 4                             # chunks per tree group
NGROUP = NCHUNK // CPG              # 2 groups per full block
GJ = CPG * JCHUNK                   # 128 j's (= (c,q,j) groups) per tree
DIRECT_EVERY = 5                    # every Nth GROUP takes the DVE-direct path

_cache = {}


def _build_program():
    import concourse.mybir as mybir
    from concourse import bacc, tile
    from concourse.masks import make_identity

    f32 = mybir.dt.float32
    bf16 = mybir.dt.bfloat16
    fp8 = mybir.dt.float8e4
    Alu = mybir.AluOpType
    Act = mybir.ActivationFunctionType

    nc = bacc.Bacc("TRN2", target_bir_lowering=False, debug=False,
                   enable_asserts=False)

    xT_d = nc.dram_tensor("xT", [IN_FEATURES, B], fp8, kind="ExternalInput").ap()
    T_d = nc.dram_tensor("Tl", [IN_FEATURES, N_LOC], fp8, kind="ExternalInput").ap()
    rp_d = nc.dram_tensor("rp", [K + 2, K * B], fp8,
                          kind="ExternalInput").ap()
    feat_d = nc.dram_tensor("feat", [B, O_LOC], f32, kind="ExternalOutput").ap()

    JK = K * B                      # 12800 diff columns per full block
    CH = QB * 512                   # 2048 PSUM elements per chunk (1600 used)
    # ba scratch: 4-chunk level-0 (128 groups x 50) + tree level regions
    BA_COLS = 12672

    with tile.TileContext(nc) as tc:
        with (
            tc.tile_pool(name="static", bufs=1) as static,
            tc.tile_pool(name="babsp", bufs=3) as babsp,
            tc.tile_pool(name="dexpp", bufs=2) as dexpp,
            tc.tile_pool(name="et0p", bufs=8) as et0p,
            tc.tile_pool(name="et1p", bufs=2) as et1p,
            tc.tile_pool(name="dramp", bufs=1, space="DRAM") as dramp,
        ):
            # ---- rhs I-part loads first: they gate the pairwise stage ----
            rhs_t = []
            for h in range(2):
                rt = static.tile([K + 1, JK], fp8, tag=f"rhs{h}",
                                 name=f"rhs{h}")
                nc.sync.dma_start(out=rt[:, 0:JK // 2],
                                  in_=rp_d[0:K + 1, 0:JK // 2])
                nc.scalar.dma_start(out=rt[:, JK // 2:],
                                    in_=rp_d[0:K + 1, JK // 2:])
                rhs_t.append(rt)

            # ---- stage 1: load inputs, M = x @ T_local ---------------------
            xt_sb = static.tile([P, CC * B], fp8, tag="xt")
            t_sb = static.tile([P, CC * N_LOC], fp8, tag="t")
            for cc in range(CC):
                nc.sync.dma_start(out=xt_sb[:, cc * B:(cc + 1) * B],
                                  in_=xT_d[cc * P:(cc + 1) * P, :])
                nc.scalar.dma_start(out=t_sb[:, cc * N_LOC:(cc + 1) * N_LOC],
                                    in_=T_d[cc * P:(cc + 1) * P, :])

            warm = static.tile([1, 2], f32, tag="warm")
            nc.vector.memset(warm[:, :], 0.0)
            nc.scalar.activation(out=warm[:, :], in_=warm[:, :],
                                 func=Act.Exp, scale=-1.0)
            ident = static.tile([P, P], bf16, tag="ident")
            make_identity(nc, ident[:, :])
            identf = static.tile([JBANK, JBANK], f32, tag="identf")
            make_identity(nc, identf[:, :])
            ones_col = static.tile([P, 1], f32, tag="ones_col")
            nc.vector.memset(ones_col[:, :], 1.0)

            # -M staged to DRAM as one flat j-major row per o, so the
            # per-o rhs row refresh is a single contiguous 25.6KB packet
            negm_d = dramp.tile([O_LOC, K * B], fp8, tag="negm_d")
            m_bf = []
            m_bb = []
            ngs = []
            with tc.tile_pool(name="mmp", bufs=2, space="PSUM") as mmp:
                for it in range(ITILES):
                    pm = mmp.tile([P, N_LOC], f32, tag="pm")
                    for cc in range(CC):
                        nc.tensor.matmul(
                            pm[:, :],
                            lhsT=xt_sb[:, cc * B + it * P: cc * B + it * P + P],
                            rhs=t_sb[:, cc * N_LOC:(cc + 1) * N_LOC],
                            start=(cc == 0), stop=(cc == CC - 1),
                        )
                    mb = static.tile([P, N_LOC], fp8, tag=f"mbf{it}",
                                     name=f"mbf{it}")
                    nc.scalar.copy(mb[:, :], pm[:, :])
                    m_bf.append(mb)
                    mbb = static.tile([P, N_LOC], bf16, tag=f"mbb{it}",
                                      name=f"mbb{it}")
                    nc.scalar.copy(mbb[:, :], mb[:, :])
                    m_bb.append(mbb)
                    ng = static.tile([P, N_LOC], fp8, tag=f"neg{it}",
                                     name=f"neg{it}")
                    nc.vector.tensor_scalar(out=ng[:, :], in0=mb[:, :],
                                            scalar1=-1.0, scalar2=None,
                                            op0=Alu.mult)
                    ngs.append(ng)
                half = K * P
                for o in range(O_LOC):
                    for it in range(ITILES):
                        nc.sync.dma_start(
                            out=negm_d[o:o + 1,
                                       it * half:(it + 1) * half],
                            in_=ngs[it][:, o * K:(o + 1) * K])

            # ---- stage 2: lhsT tiles  [M_o^T (50 rows); ones] --------------
            # lhsT tiles [M_o^T (50 rows); ones] — ones row arrives by
            # DMA from rp row 51 (partition 50 is not engine-alignable)
            lhs = []
            with tc.tile_pool(name="tpp", bufs=2, space="PSUM") as tpp:
                for o in range(O_LOC):
                    lt = static.tile([K + 1, B], fp8, tag=f"lhs{o}",
                                     name=f"lhs{o}")
                    for it in range(ITILES):
                        tp = tpp.tile([K, P], bf16, tag="tp")
                        nc.tensor.transpose(
                            tp[:, :], m_bb[it][:, o * K: o * K + K],
                            ident[:, :])
                        nc.scalar.copy(lt[0:K, it * P:(it + 1) * P], tp[:, :])
                    nc.sync.dma_start(out=lt[K:K + 1, 0:B],
                                      in_=rp_d[K + 1:K + 2, 0:B])
                    lhs.append(lt)

            # ---- stage 4: per (o, itile): diffs -> |.| -> k-sum -> exp -----
            feat_sb = [static.tile([P, O_LOC], f32, tag=f"feat{it}",
                                   name=f"feat{it}")
                       for it in range(ITILES)]
            et0_tiles = []
            group_idx = 0
            stage4 = tc.tile_pool(name="chp", bufs=2, space="PSUM")
            chp = stage4.__enter__()
            for o in range(O_LOC):
                rt = rhs_t[o % 2]
                nc.sync.dma_start(out=rt[K:K + 1, :],
                                  in_=negm_d[o:o + 1, :])
                for it in range(ITILES):
                    g_lo = 0 if it == 0 else NGROUP // 2
                    nj = (NGROUP - g_lo) * GJ
                    dexp = dexpp.tile([P, B], f32, tag="dexp")
                    for g in range(g_lo, NGROUP):
                        direct = group_idx % DIRECT_EVERY == 0
                        gsl = dexp[:, (g - g_lo) * GJ:(g - g_lo + 1) * GJ]
                        if not direct:
                            ba = babsp.tile([P, BA_COLS], bf16, tag="ba")
                        for cc in range(CPG):
                            c = g * CPG + cc
                            ch = chp.tile([P, CH], f32, tag="ch")
                            for q in range(QB):
                                col = (c * JCHUNK + q * JBANK) * K
                                nc.tensor.matmul(
                                    ch[:, q * 512: q * 512 + JBANK * K],
                                    lhsT=lhs[o][:, it * P:(it + 1) * P],
                                    rhs=rt[:, col: col + JBANK * K],
                                    start=True, stop=True)
                            # PSUM chunk viewed [p, q(4), j(8), k(50)]
                            ch4 = ch[:, :].rearrange(
                                "p (q r) -> p q r", q=QB)[
                                :, :, 0:JBANK * K].rearrange(
                                "p q (j k) -> p q j k", k=K)
                            if direct:
                                # DVE: fused |.| + k-reduce from PSUM
                                nc.vector.tensor_reduce(
                                    out=gsl[:, cc * JCHUNK:
                                            (cc + 1) * JCHUNK].rearrange(
                                        "p (q j) -> p q j", q=QB),
                                    in_=ch4,
                                    axis=mybir.AxisListType.X, op=Alu.add,
                                    apply_absolute_value=True)
                            else:
                                # ScalarE |.| cast to bf16 (dense j-major)
                                nc.scalar.activation(
                                    out=ba[:, cc * JCHUNK * K:
                                           (cc + 1) * JCHUNK * K].rearrange(
                                        "p (q j k) -> p q j k",
                                        q=QB, j=JBANK),
                                    in_=ch4, func=Act.Abs)
                        if not direct:
                            # group tree: 128 (c,q,j) groups x k, dense
                            # even-width halvings at DVE 2x; odd leftovers
                            # pair-added into 1-wide regions
                            def view(ofs, width):
                                return ba[:, ofs: ofs + GJ * width].\
                                    rearrange("p (g k) -> p g k", k=width)
                            cur, w = 0, K
                            free = GJ * K
                            singles = []
                            while w > 1:
                                hw = w // 2
                                if hw > 1 and hw % 2:
                                    hw -= 1
                                src = view(cur, w)
                                rem = w - 2 * hw
                                if rem == 1:
                                    singles.append(src[:, :, w - 1:w])
                                elif rem == 2:
                                    nc.vector.tensor_tensor(
                                        out=view(free, 1),
                                        in0=src[:, :, w - 2:w - 1],
                                        in1=src[:, :, w - 1:w],
                                        op=Alu.add)
                                    singles.append(view(free, 1))
                                    free += GJ
                                nc.vector.tensor_tensor(
                                    out=view(free, hw),
                                    in0=src[:, :, 0:hw],
                                    in1=src[:, :, hw:2 * hw],
                                    op=Alu.add)
                                cur = free
                                free += hw * GJ
                                w = hw
                            gsl3 = gsl.rearrange("p (g k) -> p g k", k=1)
                            for si, sv in enumerate(singles):
                                last = si == len(singles) - 1
                                dst = gsl3 if last else view(free, 1)
                                nc.vector.tensor_tensor(
                                    out=dst, in0=view(cur, 1), in1=sv,
                                    op=Alu.add)
                                cur = free
                                free += GJ
                            if not singles:
                                nc.vector.tensor_copy(out=gsl3,
                                                      in_=view(cur, 1))
                        group_idx += 1
                    if it == 0:
                        et = et0p.tile([P, B], f32, tag="et0",
                                       name=f"et0_{o}")
                        et0_tiles.append(et)
                    else:
                        et = et1p.tile([P, B // 2], f32, tag="et1")
                    nc.scalar.activation(out=et[:, :], in_=dexp[:, 0:nj],
                                         func=Act.Exp, scale=-1.0)
                    nc.vector.tensor_reduce(
                        out=feat_sb[it][:, o:o + 1], in_=et[:, :],
                        axis=mybir.AxisListType.X, op=Alu.add)
            stage4.__exit__(None, None, None)

            # ---- stage 5: mirrored contribution for itile 1 ----------------
            # colsum_o[j] = sum_{i in it0} exp(-D[i, j]) for j in [128, 256)
            cs_sb = static.tile([JBANK, P], f32, tag="cs_sb")
            with tc.tile_pool(name="csp", bufs=2, space="PSUM") as csp:
                for o in range(O_LOC):
                    cs = csp.tile([1, P], f32, tag="cs")
                    nc.tensor.matmul(cs[:, :], lhsT=ones_col[:, :],
                                     rhs=et0_tiles[o][:, P:B],
                                     start=True, stop=True)
                    cs_row = babsp.tile([1, P], f32, tag="cs_row")
                    nc.scalar.copy(cs_row[:, :], cs[:, :])
                    nc.sync.dma_start(out=cs_sb[o:o + 1, :], in_=cs_row[:, :])
                ct = csp.tile([P, JBANK], f32, tag="ct")
                nc.tensor.transpose(ct[:, :], cs_sb[:, :], identf[:, :])
                nc.vector.tensor_tensor(out=feat_sb[1][:, :],
                                        in0=feat_sb[1][:, :],
                                        in1=ct[:, :], op=Alu.add)

            for it in range(ITILES):
                nc.vector.tensor_scalar(
                    out=feat_sb[it][:, :], in0=feat_sb[it][:, :],
                    scalar1=1.0, scalar2=None, op0=Alu.subtract)
                nc.sync.dma_start(out=feat_d[it * P:(it + 1) * P, :],
                                  in_=feat_sb[it][:, :])

    nc.compile()
    return nc


def _get_program():
    if "nc" not in _cache:
        _cache["nc"] = _build_program()
    return _cache["nc"]


def prepare_in_maps(x, T):
    """Host-side sharding: transpose/cast x, slice + K-pad T per core."""
    f8 = ml_dtypes.float8_e4m3fn
    xT = np.ascontiguousarray(np.asarray(x, dtype=np.float32).T).astype(f8)
    Tf = np.asarray(T, dtype=np.float32)
    in_maps = []
    rp = np.zeros((K + 2, K * B), dtype=ml_dtypes.float8_e4m3fn)
    kk = np.arange(K)
    for j in range(B):
        rp[kk, j * K + kk] = 1.0
    rp[K + 1, :] = 1.0
    for c in range(N_CORES):
        Tl = np.ascontiguousarray(
            Tf[:, c * N_LOC:(c + 1) * N_LOC]).astype(f8)
        in_maps.append({"xT": xT, "Tl": Tl, "rp": rp})
    return in_maps


def run_cores(in_maps, trace=False, tmpdir=None):
    from concourse import bass_utils
    nc = _get_program()
    return bass_utils.run_bass_kernel_spmd(
        nc, in_maps, core_ids=list(range(N_CORES)), trace=trace, tmpdir=tmpdir)


def kernel(x, T):
    x = np.asarray(x, dtype=np.float32)
    res = run_cores(prepare_in_maps(x, T))
    feat = np.concatenate(
        [res.results[c]["feat"].astype(np.float32) for c in range(N_CORES)],
        axis=1)
    return np.concatenate([x, feat], axis=1)


# revision 35
# speedup vs baseline: 1.1165x; 1.0031x over previous
"""Trainium2 Bass kernel for MinibatchDiscrimination.

Reference computation (B=256, IN=1024, O=64, K=50):
    M = (x @ T).reshape(B, O, K)
    l1[i,j,o] = sum_k |M[i,o,k] - M[j,o,k]|
    out = concat([x, sum_j exp(-l1) - 1], axis=1)          # [B, IN + O]

Sharding: the O (out_features) dimension is split across the 8 NeuronCores
(8 features per core); x is replicated. Each core computes its [256, 8]
feature block; the host gathers the blocks and concatenates with x.

Per-core pipeline:
  1. PE matmul: M[256, 512] = xT.T @ T_local (bf16 in, f32 PSUM; K padded to
     64 only for this GEMM), cast to bf16 — the canonical value used on BOTH
     sides of the pairwise subtraction, so the diagonal distance is exactly
     zero. -M is staged to DRAM.
  2. All-pairs signed differences are generated by the PE with an affine
     matmul: diff[i, (j,k)] = sum_p lhsT[p,i] * rhs[p,(j,k)] with
     lhsT = [M_o^T (50 k-rows); ones] and rhs = [I50 tiled over j; -M_o row].
     Chunks of 32 j land in PSUM as [128, 4x512] f32 (400 of each 512-col
     bank used: 8 j x 50 k).
  3. Exploiting D[i,j] = D[j,i], the itile-1 blocks only compute j in
     [128,256); the mirrored contribution comes from PE column-sums of the
     itile-0 exp tiles at the end.
  4. Each PSUM chunk takes one of two abs+k-reduce paths (balancing DVE and
     ScalarE): (a) DVE tensor_reduce(add, apply_absolute_value) straight
     from PSUM, or (b) ScalarE Abs-cast to bf16 SBUF (written k-major) +
     dense DVE binary-tree tensor_tensor adds at 2x.
  5. ScalarE exp(-l1) (scale=-1), DVE reduce over j, -1.0, DMA out.
"""

import numpy as np
import ml_dtypes

B = 256
IN_FEATURES = 1024
O_TOTAL = 64
K = 50
K64 = 64
N_CORES = 8
O_LOC = O_TOTAL // N_CORES          # 8 features per core
N_LOC = O_LOC * K                   # 400 M columns per core
P = 128                             # partitions
ITILES = B // P                     # 2 row tiles
CC = IN_FEATURES // P               # 8 contraction chunks
JCHUNK = 32                         # j's per PSUM chunk
JBANK = 8                           # j's per PSUM bank (8*50 = 400 of 512)
QB = JCHUNK // JBANK                # banks per chunk = 4
NCHUNK = B // JCHUNK                # 8 chunks per full block
KP = 26                             # DoubleRow partitions (2 planes of 26)
JKH = K * B                         # columns per plane
CPG = 4                             # chunks per tree group
NGROUP = NCHUNK // CPG              # 2 groups per full block
GJ = CPG * JCHUNK                   # 128 j's (= (c,q,j) groups) per tree
DIRECT_EVERY = 5                    # every Nth GROUP takes the DVE-direct path

_cache = {}


def _build_program():
    import concourse.mybir as mybir
    from concourse import bacc, tile
    from concourse.masks import make_identity

    f32 = mybir.dt.float32
    bf16 = mybir.dt.bfloat16
    fp8 = mybir.dt.float8e4
    Alu = mybir.AluOpType
    Act = mybir.ActivationFunctionType

    nc = bacc.Bacc("TRN2", target_bir_lowering=False, debug=False,
                   enable_asserts=False)

    xT_d = nc.dram_tensor("xT", [IN_FEATURES, B], fp8, kind="ExternalInput").ap()
    T_d = nc.dram_tensor("Tl", [IN_FEATURES, N_LOC], fp8, kind="ExternalInput").ap()
    # folded DoubleRow rhs prototype: 26 partitions x 2 planes
    # (k' = i*26 + p; k'<50 tiled-identity, k'=50 -M placeholder,
    # k'=51 zeros) + a ones row for the lhsT build
    rp_d = nc.dram_tensor("rp", [KP + 1, 2 * K * B], fp8,
                          kind="ExternalInput").ap()
    feat_d = nc.dram_tensor("feat", [B, O_LOC], f32, kind="ExternalOutput").ap()

    JK = K * B                      # 12800 diff columns per full block
    CH = QB * 512                   # 2048 PSUM elements per chunk (1600 used)
    # ba scratch: 4-chunk level-0 (128 groups x 50) + tree level regions
    BA_COLS = 12672

    with tile.TileContext(nc) as tc:
        with (
            tc.tile_pool(name="static", bufs=1) as static,
            tc.tile_pool(name="babsp", bufs=3) as babsp,
            tc.tile_pool(name="dexpp", bufs=2) as dexpp,
            tc.tile_pool(name="et0p", bufs=8) as et0p,
            tc.tile_pool(name="et1p", bufs=2) as et1p,
            tc.tile_pool(name="dramp", bufs=1, space="DRAM") as dramp,
        ):
            # ---- rhs I-part loads first: they gate the pairwise stage ----
            rhs_t = []
            for h in range(2):
                rt = static.tile([KP, 2 * JK], fp8, tag=f"rhs{h}",
                                 name=f"rhs{h}")
                nc.sync.dma_start(out=rt[:, 0:JK],
                                  in_=rp_d[0:KP, 0:JK])
                nc.scalar.dma_start(out=rt[:, JK:],
                                    in_=rp_d[0:KP, JK:])
                rhs_t.append(rt)

            # ---- stage 1: load inputs, M = x @ T_local ---------------------
            xt_sb = static.tile([P, CC * B], fp8, tag="xt")
            t_sb = static.tile([P, CC * N_LOC], fp8, tag="t")
            for cc in range(CC):
                nc.sync.dma_start(out=xt_sb[:, cc * B:(cc + 1) * B],
                                  in_=xT_d[cc * P:(cc + 1) * P, :])
                nc.scalar.dma_start(out=t_sb[:, cc * N_LOC:(cc + 1) * N_LOC],
                                    in_=T_d[cc * P:(cc + 1) * P, :])

            warm = static.tile([1, 2], f32, tag="warm")
            nc.vector.memset(warm[:, :], 0.0)
            nc.scalar.activation(out=warm[:, :], in_=warm[:, :],
                                 func=Act.Exp, scale=-1.0)
            ident = static.tile([P, P], bf16, tag="ident")
            make_identity(nc, ident[:, :])
            identf = static.tile([JBANK, JBANK], f32, tag="identf")
            make_identity(nc, identf[:, :])
            ones_col = static.tile([P, 1], f32, tag="ones_col")
            nc.vector.memset(ones_col[:, :], 1.0)

            # -M staged to DRAM as one flat j-major row per o, so the
            # per-o rhs row refresh is a single contiguous 25.6KB packet
            negm_d = dramp.tile([O_LOC, K * B], fp8, tag="negm_d")
            m_bf = []
            m_bb = []
            ngs = []
            with tc.tile_pool(name="mmp", bufs=2, space="PSUM") as mmp:
                for it in range(ITILES):
                    pm = mmp.tile([P, N_LOC], f32, tag="pm")
                    for cc in range(CC):
                        nc.tensor.matmul(
                            pm[:, :],
                            lhsT=xt_sb[:, cc * B + it * P: cc * B + it * P + P],
                            rhs=t_sb[:, cc * N_LOC:(cc + 1) * N_LOC],
                            start=(cc == 0), stop=(cc == CC - 1),
                        )
                    mb = static.tile([P, N_LOC], fp8, tag=f"mbf{it}",
                                     name=f"mbf{it}")
                    nc.scalar.copy(mb[:, :], pm[:, :])
                    m_bf.append(mb)
                    mbb = static.tile([P, N_LOC], bf16, tag=f"mbb{it}",
                                      name=f"mbb{it}")
                    nc.scalar.copy(mbb[:, :], mb[:, :])
                    m_bb.append(mbb)
                    ng = static.tile([P, N_LOC], fp8, tag=f"neg{it}",
                                     name=f"neg{it}")
                    nc.vector.tensor_scalar(out=ng[:, :], in0=mb[:, :],
                                            scalar1=-1.0, scalar2=None,
                                            op0=Alu.mult)
                    ngs.append(ng)
                half = K * P
                for o in range(O_LOC):
                    for it in range(ITILES):
                        nc.sync.dma_start(
                            out=negm_d[o:o + 1,
                                       it * half:(it + 1) * half],
                            in_=ngs[it][:, o * K:(o + 1) * K])

            # ---- stage 2: lhsT tiles [26, (2 planes, 256)] -----------------
            # plane i holds M_o^T rows k' = i*26 + p; (24,1)=ones, (25,1)=0
            lhs = []
            with tc.tile_pool(name="tpp", bufs=2, space="PSUM") as tpp:
                for o in range(O_LOC):
                    lt = static.tile([KP, 2 * B], fp8, tag=f"lhs{o}",
                                     name=f"lhs{o}")
                    for it in range(ITILES):
                        tp1 = tpp.tile([KP, P], bf16, tag="tp1")
                        nc.tensor.transpose(
                            tp1[:, :], m_bb[it][:, o * K: o * K + KP],
                            ident[:, :])
                        nc.scalar.copy(lt[0:KP, it * P:(it + 1) * P],
                                       tp1[:, :])
                        tp2 = tpp.tile([K - KP, P], bf16, tag="tp2")
                        nc.tensor.transpose(
                            tp2[:, :], m_bb[it][:, o * K + KP: o * K + K],
                            ident[:, :])
                        nc.scalar.copy(
                            lt[0:K - KP, B + it * P: B + (it + 1) * P],
                            tp2[:, :])
                    nc.sync.dma_start(out=lt[K - KP:K - KP + 1, B:2 * B],
                                      in_=rp_d[KP:KP + 1, 0:B])
                    nc.sync.dma_start(out=lt[KP - 1:KP, B:2 * B],
                                      in_=rp_d[KP - 1:KP, JK:JK + B])
                    lhs.append(lt)

            # ---- stage 4: per (o, itile): diffs -> |.| -> k-sum -> exp -----
            feat_sb = [static.tile([P, O_LOC], f32, tag=f"feat{it}",
                                   name=f"feat{it}")
                       for it in range(ITILES)]
            et0_tiles = []
            group_idx = 0
            stage4 = tc.tile_pool(name="chp", bufs=2, space="PSUM")
            chp = stage4.__enter__()
            for o in range(O_LOC):
                rt = rhs_t[o % 2]
                nc.sync.dma_start(out=rt[KP - 2:KP - 1, JK:2 * JK],
                                  in_=negm_d[o:o + 1, :])
                for it in range(ITILES):
                    g_lo = 0 if it == 0 else NGROUP // 2
                    nj = (NGROUP - g_lo) * GJ
                    dexp = dexpp.tile([P, B], f32, tag="dexp")
                    for g in range(g_lo, NGROUP):
                        direct = group_idx % DIRECT_EVERY == 0
                        gsl = dexp[:, (g - g_lo) * GJ:(g - g_lo + 1) * GJ]
                        if not direct:
                            ba = babsp.tile([P, BA_COLS], bf16, tag="ba")
                        for cc in range(CPG):
                            c = g * CPG + cc
                            ch = chp.tile([P, CH], f32, tag="ch")
                            lt3 = lhs[o][:, :].rearrange(
                                "p (i m) -> p i m", i=2)[
                                :, :, it * P:(it + 1) * P]
                            rt3 = rt[:, :].rearrange(
                                "p (i c) -> p i c", i=2)
                            for q in range(QB):
                                col = (c * JCHUNK + q * JBANK) * K
                                nc.tensor.matmul(
                                    ch[:, q * 512: q * 512 + JBANK * K],
                                    lhsT=lt3,
                                    rhs=rt3[:, :, col: col + JBANK * K],
                                    start=True, stop=True,
                                    perf_mode=mybir.MatmulPerfMode.DoubleRow)
                            # PSUM chunk viewed [p, q(4), j(8), k(50)]
                            ch4 = ch[:, :].rearrange(
                                "p (q r) -> p q r", q=QB)[
                                :, :, 0:JBANK * K].rearrange(
                                "p q (j k) -> p q j k", k=K)
                            if direct:
                                # DVE: fused |.| + k-reduce from PSUM
                                nc.vector.tensor_reduce(
                                    out=gsl[:, cc * JCHUNK:
                                            (cc + 1) * JCHUNK].rearrange(
                                        "p (q j) -> p q j", q=QB),
                                    in_=ch4,
                                    axis=mybir.AxisListType.X, op=Alu.add,
                                    apply_absolute_value=True)
                            else:
                                # ScalarE |.| cast to bf16 (dense j-major)
                                nc.scalar.activation(
                                    out=ba[:, cc * JCHUNK * K:
                                           (cc + 1) * JCHUNK * K].rearrange(
                                        "p (q j k) -> p q j k",
                                        q=QB, j=JBANK),
                                    in_=ch4, func=Act.Abs)
                        if not direct:
                            # group tree: 128 (c,q,j) groups x k, dense
                            # even-width halvings at DVE 2x; odd leftovers
                            # pair-added into 1-wide regions
                            def view(ofs, width):
                                return ba[:, ofs: ofs + GJ * width].\
                                    rearrange("p (g k) -> p g k", k=width)
                            cur, w = 0, K
                            free = GJ * K
                            singles = []
                            while w > 1:
                                hw = w // 2
                                if hw > 1 and hw % 2:
                                    hw -= 1
                                src = view(cur, w)
                                rem = w - 2 * hw
                                if rem == 1:
                                    singles.append(src[:, :, w - 1:w])
                                elif rem == 2:
                                    nc.vector.tensor_tensor(
                                        out=view(free, 1),
                                        in0=src[:, :, w - 2:w - 1],
                                        in1=src[:, :, w - 1:w],
                                        op=Alu.add)
                                    singles.append(view(free, 1))
                                    free += GJ
                                nc.vector.tensor_tensor(
                                    out=view(free, hw),
                                    in0=src[:, :, 0:hw],
                                    in1=src[:, :, hw:2 * hw],
                                    op=Alu.add)
                                cur = free
                                free += hw * GJ
                                w = hw
                            gsl3 = gsl.rearrange("p (g k) -> p g k", k=1)
                            for si, sv in enumerate(singles):
                                last = si == len(singles) - 1
                                dst = gsl3 if last else view(free, 1)
                                nc.vector.tensor_tensor(
                                    out=dst, in0=view(cur, 1), in1=sv,
                                    op=Alu.add)
                                cur = free
                                free += GJ
                            if not singles:
                                nc.vector.tensor_copy(out=gsl3,
                                                      in_=view(cur, 1))
                        group_idx += 1
                    if it == 0:
                        et = et0p.tile([P, B], f32, tag="et0",
                                       name=f"et0_{o}")
                        et0_tiles.append(et)
                    else:
                        et = et1p.tile([P, B // 2], f32, tag="et1")
                    nc.scalar.activation(out=et[:, :], in_=dexp[:, 0:nj],
                                         func=Act.Exp, scale=-1.0)
                    nc.vector.tensor_reduce(
                        out=feat_sb[it][:, o:o + 1], in_=et[:, :],
                        axis=mybir.AxisListType.X, op=Alu.add)
            stage4.__exit__(None, None, None)

            # ---- stage 5: mirrored contribution for itile 1 ----------------
            # colsum_o[j] = sum_{i in it0} exp(-D[i, j]) for j in [128, 256)
            cs_sb = static.tile([JBANK, P], f32, tag="cs_sb")
            with tc.tile_pool(name="csp", bufs=2, space="PSUM") as csp:
                for o in range(O_LOC):
                    cs = csp.tile([1, P], f32, tag="cs")
                    nc.tensor.matmul(cs[:, :], lhsT=ones_col[:, :],
                                     rhs=et0_tiles[o][:, P:B],
                                     start=True, stop=True)
                    cs_row = babsp.tile([1, P], f32, tag="cs_row")
                    nc.scalar.copy(cs_row[:, :], cs[:, :])
                    nc.sync.dma_start(out=cs_sb[o:o + 1, :], in_=cs_row[:, :])
                ct = csp.tile([P, JBANK], f32, tag="ct")
                nc.tensor.transpose(ct[:, :], cs_sb[:, :], identf[:, :])
                nc.vector.tensor_tensor(out=feat_sb[1][:, :],
                                        in0=feat_sb[1][:, :],
                                        in1=ct[:, :], op=Alu.add)

            for it in range(ITILES):
                nc.vector.tensor_scalar(
                    out=feat_sb[it][:, :], in0=feat_sb[it][:, :],
                    scalar1=1.0, scalar2=None, op0=Alu.subtract)
                nc.sync.dma_start(out=feat_d[it * P:(it + 1) * P, :],
                                  in_=feat_sb[it][:, :])

    nc.compile()
    return nc


def _get_program():
    if "nc" not in _cache:
        _cache["nc"] = _build_program()
    return _cache["nc"]


def prepare_in_maps(x, T):
    """Host-side sharding: transpose/cast x, slice + K-pad T per core."""
    f8 = ml_dtypes.float8_e4m3fn
    xT = np.ascontiguousarray(np.asarray(x, dtype=np.float32).T).astype(f8)
    Tf = np.asarray(T, dtype=np.float32)
    in_maps = []
    # folded DoubleRow prototype: k' = i*26 + p; value delta(col_k == k')
    rp = np.zeros((KP + 1, 2 * K * B), dtype=ml_dtypes.float8_e4m3fn)
    for kp in range(K):
        i, p = divmod(kp, KP)
        jj = np.arange(B)
        rp[p, i * K * B + jj * K + kp] = 1.0
    rp[KP, :] = 1.0
    for c in range(N_CORES):
        Tl = np.ascontiguousarray(
            Tf[:, c * N_LOC:(c + 1) * N_LOC]).astype(f8)
        in_maps.append({"xT": xT, "Tl": Tl, "rp": rp})
    return in_maps


def run_cores(in_maps, trace=False, tmpdir=None):
    from concourse import bass_utils
    nc = _get_program()
    return bass_utils.run_bass_kernel_spmd(
        nc, in_maps, core_ids=list(range(N_CORES)), trace=trace, tmpdir=tmpdir)


def kernel(x, T):
    x = np.asarray(x, dtype=np.float32)
    res = run_cores(prepare_in_maps(x, T))
    feat = np.concatenate(
        [res.results[c]["feat"].astype(np.float32) for c in range(N_CORES)],
        axis=1)
    return np.concatenate([x, feat], axis=1)
